# revision 7
# baseline (speedup 1.0000x reference)
import os
# Keep fp32 matmuls in fp32 — the default auto-cast to bf16 flips top-k
# edge selections and blows the error budget.
os.environ["NEURON_CC_FLAGS"] = os.environ.get("NEURON_CC_FLAGS", "") + " --auto-cast=none"

import numpy as np
import jax
import jax.numpy as jnp
from functools import partial

jax.config.update("jax_default_matmul_precision", "highest")

# Problem constants (hardcoded; kernel.py must be self-contained)
B, T, L, I, H = 8, 1024, 4, 4, 768
K, HEADS, NLAYERS = 3, 4, 3
DHEAD = H // HEADS
NEG_SLOPE = 0.2
N = T + L + I  # 1032 nodes per sample
NEG = -1e30


def _static_mask():
    # Dense adjacency mask A[dst, src] for the static edges:
    # text chain (tridiagonal incl. self loops), image<->label full,
    # image<->image full (off-diag + self), label<->label full.
    A = np.zeros((N, N), dtype=np.float32)
    tt = np.eye(T, dtype=np.float32)
    tt += np.eye(T, k=1, dtype=np.float32) + np.eye(T, k=-1, dtype=np.float32)
    A[:T, :T] = tt
    # label/image block: all pairs present (off-diagonal + self loops,
    # plus full bipartite image<->label in both directions)
    A[T:, T:] = 1.0
    return A

_A_STATIC = _static_mask()


def _split(a):
    hi = a.astype(jnp.bfloat16).astype(jnp.float32)
    return hi, a - hi


def _mm3(a, b):
    # fp32-accurate matmul that survives the compiler's bf16 auto-cast:
    # hi parts are exactly bf16-representable, so downcasting is lossless.
    ah, al = _split(a)
    bh, bl = _split(b)
    return ah @ bh + (ah @ bl + al @ bh)


def _cos(a, b):
    num = _mm3(a, b.T)
    den = jnp.linalg.norm(a, axis=-1)[:, None] * jnp.linalg.norm(b, axis=-1)[None, :]
    return num / jnp.maximum(den, 1e-8)


def _topk_mask(text, other, n_other):
    # mask[t, j] = 1 iff j is among top-K cosine matches of token t
    c = _cos(text, other)                      # [T, n_other]
    _, idx = jax.lax.top_k(c, K)               # [T, K]
    return jax.nn.one_hot(idx, n_other, dtype=jnp.float32).sum(axis=1)  # [T, n_other]


def _gat_dense(x, A, W, a_s, a_d, b):
    # Dense-mask PyG GATConv (concat heads): softmax over incoming edges per dst
    h = _mm3(x, W).reshape(N, HEADS, DHEAD)
    es = (h * a_s).sum(-1)                     # [N, heads] (src term)
    ed = (h * a_d).sum(-1)                     # [N, heads] (dst term)
    e = es[None, :, :] + ed[:, None, :]        # [N(dst), N(src), heads]
    e = jnp.where(e >= 0, e, NEG_SLOPE * e)    # leaky_relu
    logits = jnp.where(A[:, :, None] > 0, e, NEG)
    m = logits.max(axis=1, keepdims=True)
    ex = jnp.exp(logits - m)
    alpha = ex / ex.sum(axis=1, keepdims=True)           # [N, N, heads]
    ah, al = _split(alpha)
    hh, hl = _split(h)
    ein = lambda p, q: jnp.einsum('dsh,shk->dhk', p, q)
    out = ein(ah, hh) + (ein(ah, hl) + ein(al, hh))      # [N, heads, dhead]
    return out.reshape(N, H) + b


def _ln(x, g, b):
    mu = x.mean(-1, keepdims=True)
    v = ((x - mu) ** 2).mean(-1, keepdims=True)
    return (x - mu) / jnp.sqrt(v + 1e-5) * g + b


def _single(t, l, im, W, att_src, att_dst, bias, ln_g, ln_b, A_static):
    x = jnp.concatenate([t, l, im], axis=0)    # [N, H]
    ml = _topk_mask(t, l, L)                   # [T, L]
    mi = _topk_mask(t, im, I)                  # [T, I]
    A = A_static
    A = A.at[:T, T:T + L].set(ml)              # label -> text (el -> ti edges)
    A = A.at[T:T + L, :T].set(ml.T)            # text -> label
    A = A.at[:T, T + L:].set(mi)               # image -> text
    A = A.at[T + L:, :T].set(mi.T)             # text -> image
    h = x
    for li in range(NLAYERS):
        res = h
        o = jax.nn.relu(_gat_dense(h, A, W[li], att_src[li], att_dst[li], bias[li]))
        h = _ln(o + res, ln_g[li], ln_b[li])
    return h[:T]


@partial(jax.pmap, in_axes=(0, 0, 0, None, None, None, None, None, None, None))
def _pmapped_v4(t, l, im, W, att_src, att_dst, bias, ln_g, ln_b, A_static):
    return _single(t, l, im, W, att_src, att_dst, bias, ln_g, ln_b, A_static)


def kernel(text_repr, label_repr, image_repr, W, att_src, att_dst, bias, ln_g, ln_b):
    # Pure data parallel over batch: one sample per NeuronCore (B == 8 cores),
    # small GAT/LN params replicated.
    t = np.asarray(text_repr, dtype=np.float32)
    l = np.asarray(label_repr, dtype=np.float32)
    im = np.asarray(image_repr, dtype=np.float32)
    W = np.asarray(W, dtype=np.float32)
    att_src = np.asarray(att_src, dtype=np.float32)
    att_dst = np.asarray(att_dst, dtype=np.float32)
    bias = np.asarray(bias, dtype=np.float32)
    ln_g = np.asarray(ln_g, dtype=np.float32)
    ln_b = np.asarray(ln_b, dtype=np.float32)
    try:
        out = _pmapped_v4(t, l, im, W, att_src, att_dst, bias, ln_g, ln_b, _A_STATIC)
        return np.asarray(out, dtype=np.float32)
    except Exception:
        # Fallback: same math on host if device execution is unavailable
        f = jax.vmap(_single, in_axes=(0, 0, 0, None, None, None, None, None, None, None))
        with jax.default_device(jax.devices('cpu')[0]):
            out = f(jnp.asarray(t), jnp.asarray(l), jnp.asarray(im), jnp.asarray(W),
                    jnp.asarray(att_src), jnp.asarray(att_dst), jnp.asarray(bias),
                    jnp.asarray(ln_g), jnp.asarray(ln_b), jnp.asarray(_A_STATIC))
        return np.asarray(out, dtype=np.float32)
